# Initial kernel scaffold
#
"""Trainium2 Bass kernel: LayerNorm + QKV projection + RoPE (dense transformer).

Full inputs in, full outputs out. Internally shards the 8192 token rows
(b=2 x n=4096) across 8 NeuronCores (data parallel, 1024 tokens/core).

Per-core pipeline:
  1. DMA x tile [128, 2048]; LayerNorm stats via bn_stats/bn_aggr;
     xn = (x - mu) * rsqrt(var + eps)          (VectorE)
  2. PE-transpose xn 128x128 blocks -> PSUM; ScalarE Identity-copy to SBUF
     casting to the matmul dtype and applying gamma/beta (per-partition
     scale/bias in transposed layout)
  3. QKV matmuls (fp16 by default: ~tf32 accuracy at 2-byte cost):
     out[t, e] accumulated over 16 k-tiles in PSUM; weights streamed as
     half-matrix chunks [128, 16, 1024], double buffered
  4. RoPE on q (VectorE) / k (GPSIMD) with host-precomputed cos/sin tables
  5. DMA out contiguous row blocks; host re-assembles [b, h, n, hd]
"""

import os
from contextlib import ExitStack

import numpy as np

import concourse.bass as bass
import concourse.tile as tile
from concourse import bacc, mybir
from concourse.bass_utils import run_bass_kernel_spmd
from concourse.masks import make_identity

# Problem shapes (hardcoded per contract)
B, N, DM = 2, 4096, 2048
NCORES = 8
TOK = B * N            # 8192 total token rows
TPC = TOK // NCORES    # 1024 tokens per core
P = 128
MT = TPC // P          # 8 m-tiles per core
KT = DM // P           # 16 k-tiles (contraction)
HEADS, HD = 16, 128
ECW = 1024             # weight-chunk width (half the e range)
NCH = DM // ECW        # 2 chunks
NB = ECW // 512        # matmul n-slices per chunk (PSUM bank = 512 fp32)
HPC = ECW // HD        # heads per chunk = 8
LN_EPS = 1e-5
ROPE_BASE = 10000.0

F32 = mybir.dt.float32
# Matmul input dtype: float16 (default; ~tf32 accuracy) or bfloat16 (faster,
# lower accuracy). Must be a 2-byte dtype.
MM_DT = getattr(mybir.dt, os.environ.get("QKV_MM_DT", "float16"))

_CACHE = {}


def _build_nc(body_reps=None):
    if body_reps is None:
        body_reps = int(os.environ.get("QKV_BODY_REPS", "1"))
    nc = bacc.Bacc("TRN2", target_bir_lowering=False, debug=False,
                   enable_asserts=False, num_devices=NCORES)

    x = nc.dram_tensor("x", [TPC, DM], F32, kind="ExternalInput").ap()
    wts = [
        nc.dram_tensor(f"w{n}", [KT, P, DM], MM_DT,
                       kind="ExternalInput").ap()
        for n in "qkv"
    ]
    gammaT = nc.dram_tensor("gammaT", [P, KT], F32, kind="ExternalInput").ap()
    betaT = nc.dram_tensor("betaT", [P, KT], F32, kind="ExternalInput").ap()
    cosT = nc.dram_tensor("cosT", [P, MT, HD // 2], F32, kind="ExternalInput").ap()
    sinT = nc.dram_tensor("sinT", [P, MT, HD // 2], F32, kind="ExternalInput").ap()
    outs = [
        nc.dram_tensor(f"{n}_out", [TPC, DM], F32, kind="ExternalOutput").ap()
        for n in "qkv"
    ]

    with tile.TileContext(nc) as tc:
        for _rep in range(body_reps):
            with ExitStack() as ctx:
                _kernel_body(ctx, tc, x, wts, gammaT, betaT, cosT, sinT, outs)
    nc.compile()
    return nc


def _kernel_body(ctx, tc, x, wts, gammaT, betaT, cosT, sinT, outs):
    nc = tc.nc

    singles = ctx.enter_context(tc.tile_pool(name="singles", bufs=1))
    xpool = ctx.enter_context(tc.tile_pool(name="xpool", bufs=3))
    stats_pool = ctx.enter_context(tc.tile_pool(name="stats", bufs=4))
    xnt_pool = ctx.enter_context(tc.tile_pool(name="xnt", bufs=1))
    wt_pool = ctx.enter_context(tc.tile_pool(name="wt", bufs=2))
    stage_pool = ctx.enter_context(tc.tile_pool(name="stage", bufs=4))
    rope_pool = ctx.enter_context(tc.tile_pool(name="rope", bufs=3))
    # One shared PSUM pool (8 banks): phase A transposes + phase B accums
    psum = ctx.enter_context(tc.tile_pool(name="psum", bufs=8, space="PSUM"))

    # One-time constants
    identity = singles.tile([P, P], F32)
    make_identity(nc, identity)
    eps_t = singles.tile([P, 1], F32)
    nc.vector.memset(eps_t, LN_EPS)
    gamma_sb = singles.tile([P, KT], F32)
    nc.sync.dma_start(out=gamma_sb, in_=gammaT)
    beta_sb = singles.tile([P, KT], F32)
    nc.sync.dma_start(out=beta_sb, in_=betaT)
    cos_sb = singles.tile([P, MT, HD // 2], F32)
    nc.sync.dma_start(out=cos_sb, in_=cosT)
    sin_sb = singles.tile([P, MT, HD // 2], F32)
    nc.sync.dma_start(out=sin_sb, in_=sinT)

    # Persistent transposed normalized activations: [p=d_inner, k, t]
    xnt = xnt_pool.tile([P, KT, TPC], MM_DT)

    # ---- Phase A: LayerNorm + transpose, per m-tile ----
    for m in range(MT):
        x_t = xpool.tile([P, DM], F32)
        nc.sync.dma_start(out=x_t, in_=x[m * P:(m + 1) * P, :])

        xg = x_t.rearrange("p (g s) -> p g s", s=512)
        st = stats_pool.tile([P, 4, nc.vector.BN_STATS_DIM], F32)
        for g in range(4):
            nc.vector.bn_stats(out=st[:, g, :], in_=xg[:, g, :])
        mv = stats_pool.tile([P, nc.vector.BN_AGGR_DIM], F32)
        nc.vector.bn_aggr(out=mv, in_=st)

        # rsig = 1/sqrt(var + eps)
        rsig = stats_pool.tile([P, 1], F32)
        nc.scalar.activation(out=rsig, in_=mv[:, 1:2],
                             func=mybir.ActivationFunctionType.Sqrt,
                             bias=eps_t, scale=1.0)
        nc.vector.reciprocal(out=rsig, in_=rsig)

        # xn = (x - mu) * rsig (in place)
        nc.vector.tensor_scalar(out=x_t, in0=x_t,
                                scalar1=mv[:, 0:1], scalar2=rsig,
                                op0=mybir.AluOpType.subtract,
                                op1=mybir.AluOpType.mult)

        # Transpose each 128x128 block; apply gamma/beta + cast to MM_DT
        # during the PSUM->SBUF copy
        for k in range(KT):
            pt = psum.tile([P, 512], F32, space="PSUM", name="ps")
            nc.tensor.transpose(pt[:, 0:P], x_t[:, k * P:(k + 1) * P],
                                identity)
            nc.scalar.activation(out=xnt[:, k, m * P:(m + 1) * P],
                                 in_=pt[:, 0:P],
                                 func=mybir.ActivationFunctionType.Identity,
                                 bias=beta_sb[:, k:k + 1],
                                 scale=gamma_sb[:, k:k + 1])

    # ---- Phase B: QKV matmuls + RoPE + store ----
    for wi, (w_dram, o_dram) in enumerate(zip(wts, outs)):
        for c in range(NCH):
            w_sb = wt_pool.tile([P, KT, ECW], MM_DT)
            for k in range(KT):
                nc.sync.dma_start(out=w_sb[:, k, :],
                                  in_=w_dram[k, :, c * ECW:(c + 1) * ECW])

            for m in range(MT):
                accs = [psum.tile([P, 512], F32, space="PSUM", name="ps")
                        for _ in range(NB)]
                for k in range(KT):
                    lhsT = xnt[:, k, m * P:(m + 1) * P]
                    for n in range(NB):
                        nc.tensor.matmul(
                            accs[n], lhsT=lhsT,
                            rhs=w_sb[:, k, n * 512:(n + 1) * 512],
                            start=(k == 0), stop=(k == KT - 1),
                        )

                stg = stage_pool.tile([P, ECW], F32)
                for n in range(NB):
                    nc.scalar.activation(
                        out=stg[:, n * 512:(n + 1) * 512], in_=accs[n],
                        func=mybir.ActivationFunctionType.Copy)

                if wi < 2:  # rope on q and k
                    eng = nc.vector if wi == 0 else nc.gpsimd
                    ov = stg.rearrange("p (h d) -> p h d", d=HD)
                    q1 = ov[:, :, 0:HD // 2]
                    q2 = ov[:, :, HD // 2:HD]
                    cos_m = cos_sb[:, m, :]
                    sin_m = sin_sb[:, m, :]
                    cos_b = bass.AP(tensor=cos_m.tensor, offset=cos_m.offset,
                                    ap=[cos_m.ap[0], [0, HPC], cos_m.ap[1]])
                    sin_b = bass.AP(tensor=sin_m.tensor, offset=sin_m.offset,
                                    ap=[sin_m.ap[0], [0, HPC], sin_m.ap[1]])
                    ta = rope_pool.tile([P, HPC, HD // 2], F32,
                                        name=f"ropeA{wi}")
                    tb = rope_pool.tile([P, HPC, HD // 2], F32,
                                        name=f"ropeB{wi}")
                    eng.tensor_mul(ta, q1, sin_b)      # A = q1*sin
                    eng.tensor_mul(tb, q2, sin_b)      # B = q2*sin
                    eng.tensor_mul(q1, q1, cos_b)      # q1 = q1*cos
                    eng.tensor_sub(q1, q1, tb)         # q1 -= B
                    eng.tensor_mul(q2, q2, cos_b)      # q2 = q2*cos
                    eng.tensor_add(q2, q2, ta)         # q2 += A

                nc.sync.dma_start(
                    out=o_dram[m * P:(m + 1) * P, c * ECW:(c + 1) * ECW],
                    in_=stg)


def _host_prep(x, ln_gamma, ln_beta, wq, wk, wv):
    """Shard/layout inputs. Returns per-core input maps."""
    xf = np.ascontiguousarray(x.reshape(TOK, DM), dtype=np.float32)
    wdt = mybir.dt.np(MM_DT)

    def tile_w(w):
        wt = np.asarray(w, np.float32).T  # [d, e]
        return np.ascontiguousarray(wt.reshape(KT, P, DM)).astype(wdt)

    wq_t, wk_t, wv_t = tile_w(wq), tile_w(wk), tile_w(wv)
    gammaT = np.ascontiguousarray(
        np.asarray(ln_gamma, np.float32).reshape(KT, P).T)
    betaT = np.ascontiguousarray(
        np.asarray(ln_beta, np.float32).reshape(KT, P).T)

    # Build RoPE tables with jax.numpy, matching the reference's fp32 trig
    # bit-for-bit (numpy's fp32 cos differs by ~3e-4 at large arguments).
    import jax.numpy as jnp
    inv_freq = 1.0 / (ROPE_BASE ** (jnp.arange(0, HD, 2, dtype=jnp.float32) / HD))
    t = jnp.arange(N, dtype=jnp.float32)
    freqs = jnp.einsum("i,j->ij", t, inv_freq)  # [N, 64]
    cos_full = np.asarray(jnp.cos(freqs), dtype=np.float32)
    sin_full = np.asarray(jnp.sin(freqs), dtype=np.float32)

    in_maps = []
    for c in range(NCORES):
        pos0 = (c * TPC) % N
        cos_c = np.ascontiguousarray(
            cos_full[pos0:pos0 + TPC].reshape(MT, P, HD // 2).transpose(1, 0, 2))
        sin_c = np.ascontiguousarray(
            sin_full[pos0:pos0 + TPC].reshape(MT, P, HD // 2).transpose(1, 0, 2))
        in_maps.append({
            "x": np.ascontiguousarray(xf[c * TPC:(c + 1) * TPC]),
            "wq": wq_t, "wk": wk_t, "wv": wv_t,
            "gammaT": gammaT, "betaT": betaT,
            "cosT": cos_c, "sinT": sin_c,
        })
    return in_maps


def _assemble(res_list, name):
    full = np.concatenate([res_list[c][name] for c in range(NCORES)], axis=0)
    return np.ascontiguousarray(
        full.reshape(B, N, HEADS, HD).transpose(0, 2, 1, 3))


def kernel(x, ln_gamma, ln_beta, wq, wk, wv, num_heads, _trace=False):
    assert int(num_heads) == HEADS
    in_maps = _host_prep(x, ln_gamma, ln_beta, wq, wk, wv)
    if "nc" not in _CACHE:
        _CACHE["nc"] = _build_nc()
    nc = _CACHE["nc"]
    r = run_bass_kernel_spmd(nc, in_maps, core_ids=list(range(NCORES)),
                             trace=_trace)
    if _trace:
        _CACHE["last_results"] = r
    q = _assemble(r.results, "q_out")
    k = _assemble(r.results, "k_out")
    v = _assemble(r.results, "v_out")
    return q, k, v



# revision 5
# speedup vs baseline: 2.7620x; 2.7620x over previous
"""Trainium2 Bass kernel: LayerNorm + QKV projection + RoPE (dense transformer).

Full inputs in, full outputs out. Internally shards the 8192 token rows
(b=2 x n=4096) across 8 NeuronCores (data parallel, 1024 tokens/core).

Per-core pipeline (v2 — steady-state PE-dense):
  Phase A: DMA x tile [128, 2048]; LayerNorm via bn_stats/bn_aggr (VectorE);
     PE-transpose 128x128 blocks -> PSUM; ScalarE copy to SBUF applying
     gamma/beta (per-partition scale/bias in transposed layout) + fp16 cast.
     xnt is double-buffered so the next rep's phase A overlaps this rep's
     matmuls (no PE idle at body boundaries).
  Phase B: QKV matmuls in fp16; per weight half-chunk one coarse DMA
     (32KB/partition contiguous); out[t, e] accumulated over 16 k-tiles in
     PSUM (6-bank rotation); ScalarE PSUM->SBUF; RoPE on q (VectorE) /
     k (GpSimd); one DMA per [128, 1024] output block.

Pools and constants are created once; body repetitions (used by the timing
harness) share them, so reps pipeline back-to-back without drain barriers.
"""

import os
from contextlib import ExitStack

import numpy as np

import concourse.bass as bass
import concourse.tile as tile
from concourse import bacc, mybir
from concourse.bass_utils import run_bass_kernel_spmd
from concourse.masks import make_identity

# Problem shapes (hardcoded per contract)
B, N, DM = 2, 4096, 2048
NCORES = 8
TOK = B * N            # 8192 total token rows
TPC = TOK // NCORES    # 1024 tokens per core
P = 128
MT = TPC // P          # 8 m-tiles per core
KT = DM // P           # 16 k-tiles (contraction)
HEADS, HD = 16, 128
ECW = 1024             # weight-chunk width (half the e range)
NCH = DM // ECW        # 2 chunks
NB = ECW // 512        # matmul n-slices per chunk (PSUM bank = 512 fp32)
HPC = ECW // HD        # heads per chunk = 8
LN_EPS = 1e-5
ROPE_BASE = 10000.0

F32 = mybir.dt.float32
# Matmul input dtype: float16 (default; ~tf32 accuracy) or bfloat16.
MM_DT = getattr(mybir.dt, os.environ.get("QKV_MM_DT", "float16"))

_CACHE = {}


def _build_nc(body_reps=None):
    if body_reps is None:
        body_reps = int(os.environ.get("QKV_BODY_REPS", "1"))
    nc = bacc.Bacc("TRN2", target_bir_lowering=False, debug=False,
                   enable_asserts=False, num_devices=NCORES)

    x = nc.dram_tensor("x", [TPC, DM], F32, kind="ExternalInput").ap()
    wts = [
        nc.dram_tensor(f"w{n}", [NCH, P, KT * ECW], MM_DT,
                       kind="ExternalInput").ap()
        for n in "qkv"
    ]
    gammaT = nc.dram_tensor("gammaT", [P, KT], F32, kind="ExternalInput").ap()
    betaT = nc.dram_tensor("betaT", [P, KT], F32, kind="ExternalInput").ap()
    cosT = nc.dram_tensor("cosT", [P, MT, HD // 2], F32, kind="ExternalInput").ap()
    sinT = nc.dram_tensor("sinT", [P, MT, HD // 2], F32, kind="ExternalInput").ap()
    outs = [
        nc.dram_tensor(f"{n}_out", [TPC, DM], F32, kind="ExternalOutput").ap()
        for n in "qkv"
    ]

    with tile.TileContext(nc) as tc:
        with ExitStack() as ctx:
            st = _make_state(ctx, tc, gammaT, betaT, cosT, sinT)
            for _rep in range(body_reps):
                _kernel_body(tc, st, x, wts, outs)
    nc.compile()
    return nc


def _make_state(ctx, tc, gammaT, betaT, cosT, sinT):
    """Pools + one-time constants, shared by all body reps."""
    nc = tc.nc
    st = {}
    singles = ctx.enter_context(tc.tile_pool(name="singles", bufs=1))
    st["xpool"] = ctx.enter_context(tc.tile_pool(name="xpool", bufs=3))
    st["stats"] = ctx.enter_context(tc.tile_pool(name="stats", bufs=4))
    st["xnt"] = ctx.enter_context(tc.tile_pool(name="xnt", bufs=2))
    st["wt"] = ctx.enter_context(tc.tile_pool(name="wt", bufs=2))
    st["stage"] = ctx.enter_context(tc.tile_pool(name="stage", bufs=4))
    st["rope"] = ctx.enter_context(tc.tile_pool(name="rope", bufs=3))
    # PSUM: 6 banks rotate through matmul accumulators, 2 for transposes
    st["psumB"] = ctx.enter_context(
        tc.tile_pool(name="psumB", bufs=6, space="PSUM"))
    st["psumA"] = ctx.enter_context(
        tc.tile_pool(name="psumA", bufs=2, space="PSUM"))

    identity = singles.tile([P, P], F32)
    make_identity(nc, identity)
    st["identity"] = identity
    eps_t = singles.tile([P, 1], F32)
    nc.vector.memset(eps_t, LN_EPS)
    st["eps"] = eps_t
    gamma_sb = singles.tile([P, KT], F32)
    nc.sync.dma_start(out=gamma_sb, in_=gammaT)
    st["gamma"] = gamma_sb
    beta_sb = singles.tile([P, KT], F32)
    nc.sync.dma_start(out=beta_sb, in_=betaT)
    st["beta"] = beta_sb
    cos_sb = singles.tile([P, MT, HD // 2], F32)
    nc.sync.dma_start(out=cos_sb, in_=cosT)
    st["cos"] = cos_sb
    sin_sb = singles.tile([P, MT, HD // 2], F32)
    nc.sync.dma_start(out=sin_sb, in_=sinT)
    st["sin"] = sin_sb
    return st


def _kernel_body(tc, st, x, wts, outs):
    nc = tc.nc
    identity, eps_t = st["identity"], st["eps"]
    gamma_sb, beta_sb = st["gamma"], st["beta"]
    cos_sb, sin_sb = st["cos"], st["sin"]

    # Persistent transposed normalized activations: [p=d_inner, k, t]
    xnt = st["xnt"].tile([P, KT, TPC], MM_DT)

    # ---- Phase A: LayerNorm + transpose, per m-tile ----
    for m in range(MT):
        x_t = st["xpool"].tile([P, DM], F32)
        nc.sync.dma_start(out=x_t, in_=x[m * P:(m + 1) * P, :])

        xg = x_t.rearrange("p (g s) -> p g s", s=512)
        stt = st["stats"].tile([P, 4, nc.vector.BN_STATS_DIM], F32)
        for g in range(4):
            nc.vector.bn_stats(out=stt[:, g, :], in_=xg[:, g, :])
        mv = st["stats"].tile([P, nc.vector.BN_AGGR_DIM], F32)
        nc.vector.bn_aggr(out=mv, in_=stt)

        # rsig = 1/sqrt(var + eps)
        rsig = st["stats"].tile([P, 1], F32)
        nc.scalar.activation(out=rsig, in_=mv[:, 1:2],
                             func=mybir.ActivationFunctionType.Sqrt,
                             bias=eps_t, scale=1.0)
        nc.vector.reciprocal(out=rsig, in_=rsig)

        # xn = (x - mu) * rsig (in place)
        nc.vector.tensor_scalar(out=x_t, in0=x_t,
                                scalar1=mv[:, 0:1], scalar2=rsig,
                                op0=mybir.AluOpType.subtract,
                                op1=mybir.AluOpType.mult)

        # Transpose each 128x128 block; apply gamma/beta + cast to MM_DT
        # during the PSUM->SBUF copy
        for k in range(KT):
            pt = st["psumA"].tile([P, 512], F32, space="PSUM", name="psA")
            nc.tensor.transpose(pt[:, 0:P], x_t[:, k * P:(k + 1) * P],
                                identity)
            nc.scalar.activation(out=xnt[:, k, m * P:(m + 1) * P],
                                 in_=pt[:, 0:P],
                                 func=mybir.ActivationFunctionType.Identity,
                                 bias=beta_sb[:, k:k + 1],
                                 scale=gamma_sb[:, k:k + 1])

    # ---- Phase B: QKV matmuls + RoPE + store ----
    for wi, (w_dram, o_dram) in enumerate(zip(wts, outs)):
        for c in range(NCH):
            w_sb = st["wt"].tile([P, KT * ECW], MM_DT)
            nc.sync.dma_start(out=w_sb, in_=w_dram[c])

            for m in range(MT):
                accs = [st["psumB"].tile([P, 512], F32, space="PSUM",
                                         name="psB")
                        for _ in range(NB)]
                for k in range(KT):
                    lhsT = xnt[:, k, m * P:(m + 1) * P]
                    for n in range(NB):
                        off = k * ECW + n * 512
                        nc.tensor.matmul(
                            accs[n], lhsT=lhsT,
                            rhs=w_sb[:, off:off + 512],
                            start=(k == 0), stop=(k == KT - 1),
                        )

                stg = st["stage"].tile([P, ECW], F32)
                for n in range(NB):
                    nc.scalar.activation(
                        out=stg[:, n * 512:(n + 1) * 512], in_=accs[n],
                        func=mybir.ActivationFunctionType.Copy)

                if wi < 2:  # rope on q and k
                    eng = nc.vector if wi == 0 else nc.gpsimd
                    ov = stg.rearrange("p (h d) -> p h d", d=HD)
                    q1 = ov[:, :, 0:HD // 2]
                    q2 = ov[:, :, HD // 2:HD]
                    cos_m = cos_sb[:, m, :]
                    sin_m = sin_sb[:, m, :]
                    cos_b = bass.AP(tensor=cos_m.tensor, offset=cos_m.offset,
                                    ap=[cos_m.ap[0], [0, HPC], cos_m.ap[1]])
                    sin_b = bass.AP(tensor=sin_m.tensor, offset=sin_m.offset,
                                    ap=[sin_m.ap[0], [0, HPC], sin_m.ap[1]])
                    ta = st["rope"].tile([P, HPC, HD // 2], F32,
                                         name=f"ropeA{wi}")
                    tb = st["rope"].tile([P, HPC, HD // 2], F32,
                                         name=f"ropeB{wi}")
                    eng.tensor_mul(ta, q1, sin_b)      # A = q1*sin
                    eng.tensor_mul(tb, q2, sin_b)      # B = q2*sin
                    eng.tensor_mul(q1, q1, cos_b)      # q1 = q1*cos
                    eng.tensor_sub(q1, q1, tb)         # q1 -= B
                    eng.tensor_mul(q2, q2, cos_b)      # q2 = q2*cos
                    eng.tensor_add(q2, q2, ta)         # q2 += A

                nc.sync.dma_start(
                    out=o_dram[m * P:(m + 1) * P, c * ECW:(c + 1) * ECW],
                    in_=stg)


def _host_prep(x, ln_gamma, ln_beta, wq, wk, wv):
    """Shard/layout inputs. Returns per-core input maps."""
    xf = np.ascontiguousarray(x.reshape(TOK, DM), dtype=np.float32)
    wdt = mybir.dt.np(MM_DT)

    def tile_w(w):
        wt = np.asarray(w, np.float32).T  # [d, e]
        # [NCH, P, KT*ECW]: [c, p, k*ECW+j] = wt[k*128+p, c*ECW+j]
        t = wt.reshape(KT, P, NCH, ECW).transpose(2, 1, 0, 3)
        return np.ascontiguousarray(t).reshape(NCH, P, KT * ECW).astype(wdt)

    wq_t, wk_t, wv_t = tile_w(wq), tile_w(wk), tile_w(wv)
    gammaT = np.ascontiguousarray(
        np.asarray(ln_gamma, np.float32).reshape(KT, P).T)
    betaT = np.ascontiguousarray(
        np.asarray(ln_beta, np.float32).reshape(KT, P).T)

    # Build RoPE tables with jax.numpy, matching the reference's fp32 trig
    # bit-for-bit (numpy's fp32 cos differs by ~3e-4 at large arguments).
    import jax.numpy as jnp
    inv_freq = 1.0 / (ROPE_BASE ** (jnp.arange(0, HD, 2, dtype=jnp.float32) / HD))
    t = jnp.arange(N, dtype=jnp.float32)
    freqs = jnp.einsum("i,j->ij", t, inv_freq)  # [N, 64]
    cos_full = np.asarray(jnp.cos(freqs), dtype=np.float32)
    sin_full = np.asarray(jnp.sin(freqs), dtype=np.float32)

    in_maps = []
    for c in range(NCORES):
        pos0 = (c * TPC) % N
        cos_c = np.ascontiguousarray(
            cos_full[pos0:pos0 + TPC].reshape(MT, P, HD // 2).transpose(1, 0, 2))
        sin_c = np.ascontiguousarray(
            sin_full[pos0:pos0 + TPC].reshape(MT, P, HD // 2).transpose(1, 0, 2))
        in_maps.append({
            "x": np.ascontiguousarray(xf[c * TPC:(c + 1) * TPC]),
            "wq": wq_t, "wk": wk_t, "wv": wv_t,
            "gammaT": gammaT, "betaT": betaT,
            "cosT": cos_c, "sinT": sin_c,
        })
    return in_maps


def _assemble(res_list, name):
    full = np.concatenate([res_list[c][name] for c in range(NCORES)], axis=0)
    return np.ascontiguousarray(
        full.reshape(B, N, HEADS, HD).transpose(0, 2, 1, 3))


def kernel(x, ln_gamma, ln_beta, wq, wk, wv, num_heads, _trace=False):
    assert int(num_heads) == HEADS
    in_maps = _host_prep(x, ln_gamma, ln_beta, wq, wk, wv)
    if "nc" not in _CACHE:
        _CACHE["nc"] = _build_nc()
    nc = _CACHE["nc"]
    r = run_bass_kernel_spmd(nc, in_maps, core_ids=list(range(NCORES)),
                             trace=_trace)
    if _trace:
        _CACHE["last_results"] = r
    q = _assemble(r.results, "q_out")
    k = _assemble(r.results, "k_out")
    v = _assemble(r.results, "v_out")
    return q, k, v
